# revision 1
# baseline (speedup 1.0000x reference)
"""Trainium2 Bass kernel for capsule-network dynamic routing.

Problem: u [64, 2048, 16], W [2048, 16, 1024] ->
  uhat = einsum('bni,nij->bnj', u, W)  (viewed [B, N, 32, 32])
  3 routing iterations (softmax over out-caps, squash) -> v [64, 32, 32]

Sharding: n (input capsules) split across 8 cores, 256 per core.
W slice stays SBUF-resident (bf16); uhat is recomputed on the PE each
routing pass (never materialized to HBM).  The per-iteration s-reduction
([64, 32, 32] partial sums) is AllReduced across cores.

Per-core n indexing: n = q*8 + r*2 + h  (q: 32 W-blocks, r: 4 PE row
groups, h: 2 PSUM column groups).  A "pair" is one (q, rr) with
r in {2rr, 2rr+1}: its 4 capsules live in PSUM [128 = 64h + b, 2048] so
every VE/GpSimd/ACT op runs with all 128 lanes busy.

Pipeline per pair (engines overlapped across pairs):
  PE:     8 bf16 matmuls -> psU [128, 2048] fp32 (4 uhats)
  ACT:    evacuate psU -> uh bf16; Exp(logit - max) via bias
  VE:     tmp = uh * v (bf16 2x); a = reduce_k(tmp); softmax reduces
  GpSimd: logits += a; ccx (r-half 1)
  ACT:    ccx = c expanded over k via per-partition scale (r-half 0)
  VE:     t2 = uh * ccx (bf16 2x)
  PE:     psS += I2b^T @ t2 slices (per-q batch, s-accumulation)

Host-side layouts per core (W/u cast to bf16):
  WB [32, 128, 1024]: WB[q, 16*p8+i, j] = W[q*8+p8, i, j]
  uB [128, 2048]:     uB[16*p8+i, q*64+b] = u[b, q*8+p8, i]   (pass A)
  uZ [128, 4096]:     uZ[32r+16h+i, (2q+h)*64+b] = u[b, n, i], 0 elsewhere
  I2B [128, 64]:      stacked 64x64 identities, bf16 (h/b-merge)
"""

import numpy as np

B = 64
N_FULL = 2048
D_IN = 16
N_OUT = 32
D_OUT = 32
J = N_OUT * D_OUT  # 1024
N_CORES = 8
NL = N_FULL // N_CORES  # 256 local capsules
QB = NL // 8  # 32 q-blocks

_CACHE = {}


def _pack_inputs(u, W):
    """Shard along n and build per-core SBUF-friendly layouts (bf16)."""
    import ml_dtypes
    bf = ml_dtypes.bfloat16
    I2B = np.tile(np.eye(B, dtype=np.float32), (2, 1)).astype(bf)
    in_maps = []
    for c in range(N_CORES):
        ul = u[:, c * NL:(c + 1) * NL, :]          # [64, 256, 16]
        Wl = W[c * NL:(c + 1) * NL]                # [256, 16, 1024]
        WB = np.ascontiguousarray(
            Wl.reshape(QB, 8, D_IN, J).reshape(QB, 128, J)).astype(bf)
        uB = np.ascontiguousarray(
            ul.reshape(B, QB, 8, D_IN).transpose(2, 3, 1, 0)
            .reshape(128, QB * B)).astype(bf)
        uZ = np.zeros((4, 32, 2 * QB, B), dtype=np.float32)
        un = ul.reshape(B, QB, 4, 2, D_IN)  # [b, q, r, h, i]
        for h in range(2):
            uZ[:, 16 * h:16 * h + 16, h::2, :] = un[:, :, :, h, :].transpose(2, 3, 1, 0)
        uZ = uZ.reshape(4 * 32, 2 * QB * B).astype(bf)
        in_maps.append({"WB": WB, "uB": uB, "uZ": uZ, "I2B": I2B})
    return in_maps


def _build_program():
    import concourse.bass as bass
    import concourse.tile as tile
    from concourse import bacc, mybir

    f32 = mybir.dt.float32
    bf16 = mybir.dt.bfloat16
    AF = mybir.ActivationFunctionType
    ALU = mybir.AluOpType
    AX = mybir.AxisListType

    nc = bacc.Bacc("TRN2", target_bir_lowering=False, debug=False,
                   num_devices=N_CORES)
    WB_d = nc.dram_tensor("WB", [QB, 128, J], bf16, kind="ExternalInput").ap()
    uB_d = nc.dram_tensor("uB", [128, QB * B], bf16, kind="ExternalInput").ap()
    uZ_d = nc.dram_tensor("uZ", [128, 2 * QB * B], bf16, kind="ExternalInput").ap()
    I2B_d = nc.dram_tensor("I2B", [128, B], bf16, kind="ExternalInput").ap()
    v_d = nc.dram_tensor("v_out", [B, J], f32, kind="ExternalOutput").ap()

    with tile.TileContext(nc) as tc:
        with (
            tc.tile_pool(name="wpool", bufs=1) as wpool,
            tc.tile_pool(name="state", bufs=1) as state,
            tc.tile_pool(name="scratch", bufs=2) as scratch,
            tc.tile_pool(name="smalls", bufs=2) as smalls,
            tc.tile_pool(name="psum", bufs=2, space="PSUM") as pp,
            tc.tile_pool(name="dram", bufs=2, space="DRAM") as dram,
        ):
            # --- load inputs ---
            w_tiles = []
            for q in range(QB):
                wt = wpool.tile([128, J], bf16, tag=f"w{q}")
                nc.sync.dma_start(wt[:], WB_d[q])
                w_tiles.append(wt)
            uB_t = state.tile([128, QB * B], bf16, tag="uB")
            nc.sync.dma_start(uB_t[:], uB_d[:])
            uZ_t = state.tile([128, 2 * QB * B], bf16, tag="uZ")
            nc.sync.dma_start(uZ_t[:], uZ_d[:])
            I2B_t = state.tile([128, B], bf16, tag="I2B")
            nc.sync.dma_start(I2B_t[:], I2B_d[:])

            # logits: blog[64h+b, (q*4+r)*32+o] for n = q*8+r*2+h
            blog = state.tile([128, NL // 2 * N_OUT], f32, tag="blog")
            nc.gpsimd.memset(blog[:], 0.0)
            v_t = state.tile([B, J], f32, tag="v")
            v_bf = state.tile([128, J], bf16, tag="v_bf")

            def ar_squash(merged_ps, scale):
                """merged [64,J] psum -> AllReduce -> squash -> v_t, v_bf."""
                s_loc = scratch.tile([B, J], f32, tag="st", bufs=1)
                nc.scalar.mul(s_loc[:], merged_ps[:], scale)
                bin_ = dram.tile([B, J], f32, tag="bounce_in")
                bout = dram.tile([B, J], f32, tag="bounce_out")
                nc.sync.dma_start(bin_[:], s_loc[:])
                nc.gpsimd.collective_compute(
                    "AllReduce", ALU.add,
                    replica_groups=[list(range(N_CORES))],
                    ins=[bin_.opt()], outs=[bout.opt()],
                )
                s_g = scratch.tile([B, J], f32, tag="st2", bufs=1)
                nc.sync.dma_start(s_g[:], bout[:])
                # squash: v = s * sqrt(n2)/(1+n2)
                sq = scratch.tile([B, J], f32, tag="st", bufs=1)
                nc.vector.tensor_mul(sq[:], s_g[:], s_g[:])
                n2 = smalls.tile([B, N_OUT], f32, tag="n2")
                nc.vector.reduce_sum(
                    n2[:], sq[:].rearrange("p (o k) -> p o k", k=D_OUT), axis=AX.X)
                n2p1 = smalls.tile([B, N_OUT], f32, tag="n2p1")
                nc.scalar.add(n2p1[:], n2[:], 1.0)
                rcp = smalls.tile([B, N_OUT], f32, tag="rcp")
                nc.vector.reciprocal(rcp[:], n2p1[:])
                rt = smalls.tile([B, N_OUT], f32, tag="rt")
                nc.scalar.activation(rt[:], n2[:], AF.Sqrt)
                scl = smalls.tile([B, N_OUT], f32, tag="scl")
                nc.vector.tensor_mul(scl[:], rt[:], rcp[:])
                nc.vector.tensor_mul(
                    v_t[:].rearrange("p (o k) -> p o k", k=D_OUT),
                    s_g[:].rearrange("p (o k) -> p o k", k=D_OUT),
                    scl[:].unsqueeze(2).broadcast_to([B, N_OUT, D_OUT]))
                nc.vector.tensor_copy(v_bf[0:B, :], v_t[:])
                nc.sync.dma_start(v_bf[B:2 * B, :], v_bf[0:B, :])

            # ---- pass A: s1 = (1/32) * sum_n uhat ----
            psA = pp.tile([B, J], f32, tag="uchunk", bufs=1)
            for q in range(QB):
                for jh in range(2):
                    nc.tensor.matmul(
                        psA[:, jh * 512:(jh + 1) * 512],
                        lhsT=uB_t[:, q * B:(q + 1) * B],
                        rhs=w_tiles[q][:, jh * 512:(jh + 1) * 512],
                        start=(q == 0), stop=(q == QB - 1))
            ar_squash(psA, 1.0 / N_OUT)

            # ---- passes B, C ----
            for it in range(2):
                psS = pp.tile([B, J], f32, tag="psS", bufs=1)
                for q in range(QB):
                    t2s = []
                    for rr in range(2):
                        first = (q == 0 and rr == 0)
                        ch0 = q * 4 + rr * 2
                        psU = pp.tile([128, 2 * J], f32, tag="uchunk", bufs=1)
                        for dr in range(2):
                            r = rr * 2 + dr
                            for h in range(2):
                                for jh in range(2):
                                    nc.tensor.matmul(
                                        psU[B * h:B * (h + 1),
                                            dr * J + jh * 512:
                                            dr * J + (jh + 1) * 512],
                                        lhsT=uZ_t[32 * r:32 * r + 32,
                                                  (2 * q + h) * B:
                                                  (2 * q + h + 1) * B],
                                        rhs=w_tiles[q][32 * r:32 * r + 32,
                                                       jh * 512:(jh + 1) * 512],
                                        start=True, stop=True,
                                        tile_position=(32 * r, B * h))
                        # evacuate as bf16 (ScalarE, near PSUM)
                        uh = scratch.tile([128, 2 * J], bf16, tag="uh")
                        nc.scalar.mul(uh[:], psU[:], 1.0)
                        # a[128, 64] = sum_k uhat * v   (bf16 2x mult)
                        tmp = scratch.tile([128, 2 * J], bf16, tag="tmp", bufs=1)
                        for dr in range(2):
                            nc.vector.tensor_mul(
                                tmp[:, dr * J:(dr + 1) * J],
                                uh[:, dr * J:(dr + 1) * J], v_bf[:])
                        aa = smalls.tile([128, 2 * N_OUT], f32, tag="aa")
                        nc.vector.reduce_sum(
                            aa[:], tmp[:].rearrange("p (g k) -> p g k", k=D_OUT),
                            axis=AX.X)
                        bsl = blog[:, ch0 * N_OUT:(ch0 + 2) * N_OUT]
                        nc.gpsimd.tensor_add(bsl, bsl, aa[:])
                        # softmax over o: exp(x - max) via ACT bias
                        mx = smalls.tile([128, 2], f32, tag="mx")
                        nc.vector.reduce_max(
                            mx[:], bsl.rearrange("p (g o) -> p g o", o=N_OUT),
                            axis=AX.X)
                        nmx = smalls.tile([128, 2], f32, tag="nmx")
                        nc.scalar.mul(nmx[:], mx[:], -1.0)
                        ee = smalls.tile([128, 2 * N_OUT], f32, tag="ee")
                        for dr in range(2):
                            nc.scalar.activation(
                                ee[:, dr * N_OUT:(dr + 1) * N_OUT],
                                bsl[:, dr * N_OUT:(dr + 1) * N_OUT],
                                AF.Exp, bias=nmx[:, dr:dr + 1])
                        sm = smalls.tile([128, 2], f32, tag="sm")
                        nc.vector.reduce_sum(
                            sm[:], ee[:].rearrange("p (g o) -> p g o", o=N_OUT),
                            axis=AX.X)
                        rc = smalls.tile([128, 2], f32, tag="rc")
                        nc.vector.reciprocal(rc[:], sm[:])
                        # ccx bf16 = c expanded over k; dr=0 on ACT, dr=1 gpsimd
                        ccx = scratch.tile([128, 2 * J], bf16, tag="ccx")
                        nc.scalar.mul(
                            ccx[:, 0:J].rearrange("p (o k) -> p o k", k=D_OUT),
                            ee[:, 0:N_OUT].unsqueeze(2)
                            .broadcast_to([128, N_OUT, D_OUT]),
                            rc[:, 0:1])
                        nc.gpsimd.tensor_mul(
                            ccx[:, J:2 * J].rearrange("p (o k) -> p o k", k=D_OUT),
                            ee[:, N_OUT:2 * N_OUT].unsqueeze(2)
                            .broadcast_to([128, N_OUT, D_OUT]),
                            rc[:, 1:2].unsqueeze(2)
                            .broadcast_to([128, N_OUT, D_OUT]))
                        # t2 = c * uhat (bf16 2x); merged on PE per q
                        t2 = scratch.tile([128, 2 * J], bf16, tag="t2")
                        nc.vector.tensor_mul(t2[:], uh[:], ccx[:])
                        t2s.append(t2)
                    last = (q == QB - 1)
                    for i, t2 in enumerate(t2s):
                        for sl in range(4):
                            nc.tensor.matmul(
                                psS[:, (sl % 2) * 512:(sl % 2 + 1) * 512],
                                lhsT=I2B_t[:],
                                rhs=t2[:, sl * 512:(sl + 1) * 512],
                                start=(q == 0 and i == 0 and sl < 2),
                                stop=(last and i == 1 and sl >= 2))
                ar_squash(psS, 1.0)

            nc.sync.dma_start(v_d[:], v_t[:])

    nc.compile()
    return nc


def _get_program():
    if "nc" not in _CACHE:
        _CACHE["nc"] = _build_program()
    return _CACHE["nc"]


def kernel(u, W):
    from concourse.bass_utils import run_bass_kernel_spmd

    nc = _get_program()
    in_maps = _pack_inputs(np.asarray(u, np.float32), np.asarray(W, np.float32))
    res = run_bass_kernel_spmd(nc, in_maps, list(range(N_CORES)))
    v = res.results[0]["v_out"]
    return v.reshape(B, N_OUT, D_OUT)



# revision 6
# speedup vs baseline: 1.7613x; 1.7613x over previous
"""Trainium2 Bass kernel for capsule-network dynamic routing (g-form).

Problem: u [64, 2048, 16], W [2048, 16, 1024=(o32,k32)] ->
  uhat = einsum('bni,nij->bnj', u, W), 3 routing iterations
  (softmax over out-caps o, squash over k) -> v [64, 32, 32]

Sharding: n (input capsules) split across 8 cores, NL=256 per core
(nh in {0,1} halves of 128, ns in [0,128)).

Key reformulation (avoids materializing uhat, which is elementwise-
bound): per routing iteration r>=1
    g[b,(ns,i),o]    = sum_k W[n,i,(o,k)] * v[b,(o,k)]      (PE)
    a[b,n,o]         = sum_i u[b,n,i] * g[b,(ns,i),o]       (ACT evac + VE)
    blog += a ; c = softmax_o(blog)                         (no max-sub)
    cT               = transpose_per_o(c)                   (PE transpose)
    X[ns,(i,b)]      = cT * u                               (VE/GpSimd)
    sT[(j,k),(og,b)] = sum_{nh,i,ns} W * X                  (PE, col-tiled)
This touches g-sized data (8.4M elems/core) ~3x per iteration instead
of uhat-sized (16.7M) ~5x.

s lives in "sT layout" [p=(j=o%4, k), (og=o//4, b)]; AllReduce + squash
happen in that layout (norm over k via tiny PE matmuls with block
identities J4/R4).  Final iteration: no AllReduce - partial sT3 is
DMA'd out per core and reduced + squashed on the host.

Layouts per core (all bf16):
  WSN [32=(nh,i), 128=ns, 1024=(o,k)] : W[nh*128+ns, i, (o,k)]
  WG  [128=(j,k), (og8,nh2,ns128,i16)]: W[nh*128+ns, i, (4og+j)*32+k]
  UTN [128=ns, (nh2,i16,b64)]         : u[b, nh*128+ns, i]
  UU  [128=(nh,b), (ns128,i16)]       : u[b, nh*128+ns, i]
  vv  [128=(j,k), (nh2,og8,b64)]      : v[b, 4og+j, k]  (dup over nh)
"""

import numpy as np

B = 64
N_FULL = 2048
D_IN = 16
N_OUT = 32
D_OUT = 32
J = N_OUT * D_OUT  # 1024
N_CORES = 8
NL = N_FULL // N_CORES  # 256
NH = 2
NS = 128
OG = 8  # o-groups of 4

_CACHE = {}


def _pack_inputs(u, W):
    import ml_dtypes
    bf = ml_dtypes.bfloat16
    I128 = np.eye(128, dtype=np.float32).astype(bf)
    # J4[(j,k), j'] = delta_{j,j'}; R4 = J4.T
    J4 = np.kron(np.eye(4, dtype=np.float32), np.ones((32, 1), np.float32)).astype(bf)
    R4 = np.ascontiguousarray(J4.T).astype(bf)
    in_maps = []
    for c in range(N_CORES):
        ul = u[:, c * NL:(c + 1) * NL, :]          # [64, 256, 16]
        Wl = W[c * NL:(c + 1) * NL]                # [256, 16, 1024]
        WSN = np.ascontiguousarray(
            Wl.reshape(NH, NS, D_IN, J).transpose(0, 2, 1, 3)
            .reshape(NH * D_IN, NS, J)).astype(bf)
        WG = np.ascontiguousarray(
            Wl.reshape(NH, NS, D_IN, OG, 4, 32).transpose(4, 5, 3, 0, 1, 2)
            .reshape(128, OG * NH * NS * D_IN)).astype(bf)
        UTN = np.ascontiguousarray(
            ul.reshape(B, NH, NS, D_IN).transpose(2, 1, 3, 0)
            .reshape(NS, NH * D_IN * B)).astype(bf)
        UU = np.ascontiguousarray(
            ul.reshape(B, NH, NS, D_IN).transpose(1, 0, 2, 3)
            .reshape(NH * B, NS * D_IN)).astype(bf)
        in_maps.append({"WSN": WSN, "WG": WG, "UTN": UTN, "UU": UU,
                        "I128": I128, "J4": J4, "R4": R4})
    return in_maps


def _build_program():
    import concourse.bass as bass  # noqa: F401
    import concourse.tile as tile
    from concourse import bacc, mybir

    f32 = mybir.dt.float32
    bf16 = mybir.dt.bfloat16
    AF = mybir.ActivationFunctionType
    ALU = mybir.AluOpType
    AX = mybir.AxisListType

    nc = bacc.Bacc("TRN2", target_bir_lowering=False, debug=False,
                   num_devices=N_CORES)
    WSN_d = nc.dram_tensor("WSN", [32, NS, J], bf16, kind="ExternalInput").ap()
    WG_d = nc.dram_tensor("WG", [128, OG * NH * NS * D_IN], bf16,
                          kind="ExternalInput").ap()
    UTN_d = nc.dram_tensor("UTN", [NS, NH * D_IN * B], bf16,
                           kind="ExternalInput").ap()
    UU_d = nc.dram_tensor("UU", [NH * B, NS * D_IN], bf16,
                          kind="ExternalInput").ap()
    I128_d = nc.dram_tensor("I128", [128, 128], bf16, kind="ExternalInput").ap()
    J4_d = nc.dram_tensor("J4", [128, 4], bf16, kind="ExternalInput").ap()
    R4_d = nc.dram_tensor("R4", [4, 128], bf16, kind="ExternalInput").ap()
    v_d = nc.dram_tensor("v_out", [128, 512], f32, kind="ExternalOutput").ap()

    with tile.TileContext(nc) as tc:
        with (
            tc.tile_pool(name="wpool", bufs=1) as wpool,
            tc.tile_pool(name="state", bufs=1) as state,
            tc.tile_pool(name="scratch", bufs=2) as scratch,
            tc.tile_pool(name="smalls", bufs=2) as smalls,
            tc.tile_pool(name="psum", bufs=2, space="PSUM") as pp,
            tc.tile_pool(name="dram", bufs=2, space="DRAM") as dram,
        ):
            # ---- load inputs (WSN first: pass A is gated on it) ----
            wsn = []
            for t in range(32):
                wt = wpool.tile([NS, J], bf16, tag=f"wsn{t}")
                nc.sync.dma_start(wt[:], WSN_d[t])
                wsn.append(wt)
            utn = state.tile([NS, NH * D_IN * B], bf16, tag="utn")
            nc.sync.dma_start(utn[:], UTN_d[:])
            uu = state.tile([NH * B, NS * D_IN], bf16, tag="uu")
            nc.sync.dma_start(uu[:], UU_d[:])
            i128 = state.tile([128, 128], bf16, tag="i128")
            nc.sync.dma_start(i128[:], I128_d[:])
            j4 = state.tile([128, 4], bf16, tag="j4")
            nc.sync.dma_start(j4[:], J4_d[:])
            r4 = state.tile([4, 128], bf16, tag="r4")
            nc.sync.dma_start(r4[:], R4_d[:])
            wg = state.tile([128, OG * NH * NS * D_IN], bf16, tag="wg")
            for og in range(OG):
                sl = slice(og * NH * NS * D_IN, (og + 1) * NH * NS * D_IN)
                nc.sync.dma_start(wg[:, sl], WG_d[:, sl])

            blog = state.tile([128, N_OUT * NS], f32, tag="blog")
            nc.gpsimd.memset(blog[:], 0.0)
            vv = state.tile([128, NH * OG * B], bf16, tag="vv")

            # ---- pass A: sT1 = (1/32) * sum_n uhat  (sT layout) ----
            psA = pp.tile([128, 512], f32, tag="stj", bufs=4)
            for og in range(OG):
                for nh in range(NH):
                    for i in range(D_IN):
                        nc.tensor.matmul(
                            psA[:, og * B:(og + 1) * B],
                            lhsT=wsn[nh * D_IN + i][:, og * 128:(og + 1) * 128],
                            rhs=utn[:, (nh * D_IN + i) * B:
                                    (nh * D_IN + i + 1) * B],
                            start=(nh == 0 and i == 0),
                            stop=(nh == 1 and i == D_IN - 1))

            def ar_squash(st_parts, scale, last):
                """st_parts: list of (psum_tile, part_slice) -> evac+scale,
                AllReduce, squash into vv (sT layout)."""
                s_sb = scratch.tile([128, 512], f32, tag="s_sb", bufs=1)
                for (t, psl) in st_parts:
                    nc.scalar.mul(s_sb[psl, :], t[psl, :], scale)
                if last:
                    nc.sync.dma_start(v_d[:], s_sb[:])
                    return
                bin_ = dram.tile([128, 512], f32, tag="bounce_in")
                bout = dram.tile([128, 512], f32, tag="bounce_out")
                nc.sync.dma_start(bin_[:], s_sb[:])
                nc.gpsimd.collective_compute(
                    "AllReduce", ALU.add,
                    replica_groups=[list(range(N_CORES))],
                    ins=[bin_.opt()], outs=[bout.opt()],
                )
                sTr = scratch.tile([128, 512], f32, tag="sTr", bufs=1)
                nc.sync.dma_start(sTr[:], bout[:])
                # squash: v = s * sqrt(n2)/(1+n2), n2 = sum_k s^2
                sq = scratch.tile([128, 512], bf16, tag="sqq", bufs=1)
                nc.vector.tensor_mul(sq[:], sTr[:], sTr[:])
                n2 = pp.tile([4, 512], f32, tag="stj", bufs=4)
                nc.tensor.matmul(n2[:], lhsT=j4[:], rhs=sq[:],
                                 start=True, stop=True)
                tt = smalls.tile([4, 512], f32, tag="tt", bufs=1)
                nc.scalar.activation(tt[:], n2[:], AF.Sqrt)
                dd = smalls.tile([4, 512], f32, tag="dd", bufs=1)
                nc.scalar.add(dd[:], n2[:], 1.0)
                rr_ = smalls.tile([4, 512], f32, tag="rr", bufs=1)
                nc.vector.reciprocal(rr_[:], dd[:])
                scl = smalls.tile([4, 512], bf16, tag="scl", bufs=1)
                nc.vector.tensor_mul(scl[:], tt[:], rr_[:])
                rep = pp.tile([128, 512], f32, tag="stj", bufs=4)
                nc.tensor.matmul(rep[:], lhsT=r4[:], rhs=scl[:],
                                 start=True, stop=True)
                for nh in range(NH):
                    nc.vector.tensor_mul(
                        vv[:, nh * 512:(nh + 1) * 512], sTr[:], rep[:])

            ar_squash([(psA, slice(0, 128))], 1.0 / N_OUT, last=False)

            # ---- routing iterations ----
            for r in (1, 2):
                # phase A: g + agreement -> blog
                for o in range(N_OUT):
                    og, jj = divmod(o, 4)
                    a_o = smalls.tile([128, NS], f32, tag="a_o")
                    for h in range(2):  # (ns,i)-halves of 1024 cols
                        g = pp.tile([128, 1024], f32, tag="g", bufs=2)
                        for cc in range(2):
                            for nh in range(NH):
                                lhs = vv[32 * jj:32 * jj + 32,
                                         nh * 512 + og * B:
                                         nh * 512 + (og + 1) * B]
                                base = (og * NH * NS * D_IN
                                        + nh * NS * D_IN + h * 1024 + cc * 512)
                                nc.tensor.matmul(
                                    g[B * nh:B * (nh + 1),
                                      cc * 512:(cc + 1) * 512],
                                    lhsT=lhs,
                                    rhs=wg[32 * jj:32 * jj + 32,
                                           base:base + 512],
                                    start=True, stop=True,
                                    tile_position=(32 * jj, B * nh))
                        # evac on ACT (bf16), 2x mul + 2x reduce on VE
                        ge = scratch.tile([128, 1024], bf16, tag="ge")
                        nc.scalar.mul(ge[:], g[:], 1.0)
                        th = scratch.tile([128, 1024], bf16, tag="th")
                        nc.vector.tensor_mul(
                            th[:], uu[:, h * 1024:(h + 1) * 1024], ge[:])
                        nc.vector.reduce_sum(
                            a_o[:, h * B:(h + 1) * B],
                            th[:].rearrange("p (ns i) -> p ns i", i=D_IN),
                            axis=AX.X)
                    nc.gpsimd.tensor_add(
                        blog[:, o * NS:(o + 1) * NS],
                        blog[:, o * NS:(o + 1) * NS], a_o[:])

                # phase B: softmax over o (no max-subtraction: |logits|<~1)
                ee = state.tile([128, N_OUT * NS], bf16, tag="ee")
                nc.scalar.activation(ee[:], blog[:], AF.Exp)
                ssum = smalls.tile([128, NS], f32, tag="ssum")
                nc.vector.reduce_sum(
                    ssum[:], ee[:].rearrange("p (o ns) -> p ns o", ns=NS),
                    axis=AX.X)
                rc = smalls.tile([128, NS], bf16, tag="rc")
                with nc.allow_low_precision(reason="softmax denom, tol 2e-2"):
                    nc.vector.reciprocal(rc[:], ssum[:])
                nc.vector.tensor_mul(
                    ee[:].rearrange("p (o ns) -> p o ns", ns=NS),
                    ee[:].rearrange("p (o ns) -> p o ns", ns=NS),
                    rc[:].unsqueeze(1).broadcast_to([128, N_OUT, NS]))

                # phase C: transpose c, X = cT*u, sT-MMs (col-tiled over jj)
                last = (r == 2)
                stj = [pp.tile([128, 512], f32, tag="stj", bufs=4,
                               name=f"stj{r}_{jj}")
                       for jj in range(4)]
                for og in range(OG):
                    ct_ps = pp.tile([128, 512], bf16, tag="g", bufs=2)
                    for jj in range(4):
                        o = og * 4 + jj
                        nc.tensor.transpose(
                            ct_ps[:, jj * 128:(jj + 1) * 128],
                            ee[:, o * NS:(o + 1) * NS], i128[:])
                    ct = scratch.tile([128, 512], bf16, tag="ct")
                    nc.scalar.mul(ct[:], ct_ps[:], 1.0)
                    for nh in range(NH):
                        xts = []
                        for jj in range(4):
                            xt = scratch.tile([128, D_IN * B], bf16,
                                              tag="xt", bufs=8)
                            eng = nc.gpsimd if jj == 3 else nc.vector
                            eng.tensor_mul(
                                xt[:].rearrange("p (i b) -> p i b", b=B),
                                utn[:, nh * D_IN * B:(nh + 1) * D_IN * B]
                                .rearrange("p (i b) -> p i b", b=B),
                                ct[:, jj * 128 + nh * B:jj * 128 + (nh + 1) * B]
                                .unsqueeze(1).broadcast_to([128, D_IN, B]))
                            xts.append(xt)
                        for i in range(D_IN):
                            for jj in range(4):
                                o = og * 4 + jj
                                nc.tensor.matmul(
                                    stj[jj][32 * jj:32 * jj + 32,
                                            og * B:(og + 1) * B],
                                    lhsT=wsn[nh * D_IN + i][:, o * 32:
                                                            (o + 1) * 32],
                                    rhs=xts[jj][:, i * B:(i + 1) * B],
                                    start=(nh == 0 and i == 0),
                                    stop=(nh == 1 and i == D_IN - 1),
                                    tile_position=(0, 32 * jj))
                ar_squash(
                    [(stj[jj], slice(32 * jj, 32 * jj + 32))
                     for jj in range(4)],
                    1.0, last=last)

    nc.compile()
    return nc


def _get_program():
    if "nc" not in _CACHE:
        _CACHE["nc"] = _build_program()
    return _CACHE["nc"]


def _squash_np(s, axis=-1):
    n2 = np.sum(s * s, axis=axis, keepdims=True)
    return s * (n2 / (1.0 + n2) / np.sqrt(n2))


def kernel(u, W):
    from concourse.bass_utils import run_bass_kernel_spmd

    nc = _get_program()
    in_maps = _pack_inputs(np.asarray(u, np.float32), np.asarray(W, np.float32))
    res = run_bass_kernel_spmd(nc, in_maps, list(range(N_CORES)))
    # sum partial sT3 over cores; unpack sT layout; squash on host
    sT = np.zeros((128, 512), np.float64)
    for rm in res.results:
        sT += rm["v_out"].astype(np.float64)
    # sT[(j,k), (og,b)] = s3[b, 4*og+j, k]
    s = sT.reshape(4, 32, OG, B).transpose(3, 2, 0, 1).reshape(B, N_OUT, D_OUT)
    return _squash_np(s.astype(np.float32))


# revision 7
# speedup vs baseline: 1.8330x; 1.0407x over previous
"""Trainium2 Bass kernel for capsule-network dynamic routing (g-form).

See kernel_v2_632us.py docstring for the math.  v3 changes:
  - pass A interleaved with WSN DMA ((nh,i)-outer, og-quads in 4 psum
    banks); WG DMA deferred behind pass A
  - a-reduce outputs bf16 (tries to engage faster DVE mode)
  - softmax denominator via contiguous tree-adds instead of strided reduce
  - AllReduce payload in bf16 (final output stays fp32)
  - a-mul/X-mul split across VE and GpSimd
  - dummy transposes during softmax to keep the PE HAM-warm
"""

import numpy as np

B = 64
N_FULL = 2048
D_IN = 16
N_OUT = 32
D_OUT = 32
J = N_OUT * D_OUT  # 1024
N_CORES = 8
NL = N_FULL // N_CORES  # 256
NH = 2
NS = 128
OG = 8  # o-groups of 4

_CACHE = {}


def _pack_inputs(u, W):
    import ml_dtypes
    bf = ml_dtypes.bfloat16
    I128 = np.eye(128, dtype=np.float32).astype(bf)
    J4 = np.kron(np.eye(4, dtype=np.float32), np.ones((32, 1), np.float32)).astype(bf)
    R4 = np.ascontiguousarray(J4.T).astype(bf)
    in_maps = []
    for c in range(N_CORES):
        ul = u[:, c * NL:(c + 1) * NL, :]          # [64, 256, 16]
        Wl = W[c * NL:(c + 1) * NL]                # [256, 16, 1024]
        WSN = np.ascontiguousarray(
            Wl.reshape(NH, NS, D_IN, J).transpose(0, 2, 1, 3)
            .reshape(NH * D_IN, NS, J)).astype(bf)
        WG = np.ascontiguousarray(
            Wl.reshape(NH, NS, D_IN, OG, 4, 32).transpose(4, 5, 3, 0, 1, 2)
            .reshape(128, OG * NH * NS * D_IN)).astype(bf)
        UTN = np.ascontiguousarray(
            ul.reshape(B, NH, NS, D_IN).transpose(2, 1, 3, 0)
            .reshape(NS, NH * D_IN * B)).astype(bf)
        UU = np.ascontiguousarray(
            ul.reshape(B, NH, NS, D_IN).transpose(1, 0, 2, 3)
            .reshape(NH * B, NS * D_IN)).astype(bf)
        in_maps.append({"WSN": WSN, "WG": WG, "UTN": UTN, "UU": UU,
                        "I128": I128, "J4": J4, "R4": R4})
    return in_maps


def _build_program():
    import concourse.bass as bass  # noqa: F401
    import concourse.tile as tile
    from concourse import bacc, mybir

    f32 = mybir.dt.float32
    bf16 = mybir.dt.bfloat16
    AF = mybir.ActivationFunctionType
    ALU = mybir.AluOpType
    AX = mybir.AxisListType

    nc = bacc.Bacc("TRN2", target_bir_lowering=False, debug=False,
                   num_devices=N_CORES)
    WSN_d = nc.dram_tensor("WSN", [32, NS, J], bf16, kind="ExternalInput").ap()
    WG_d = nc.dram_tensor("WG", [128, OG * NH * NS * D_IN], bf16,
                          kind="ExternalInput").ap()
    UTN_d = nc.dram_tensor("UTN", [NS, NH * D_IN * B], bf16,
                           kind="ExternalInput").ap()
    UU_d = nc.dram_tensor("UU", [NH * B, NS * D_IN], bf16,
                          kind="ExternalInput").ap()
    I128_d = nc.dram_tensor("I128", [128, 128], bf16, kind="ExternalInput").ap()
    J4_d = nc.dram_tensor("J4", [128, 4], bf16, kind="ExternalInput").ap()
    R4_d = nc.dram_tensor("R4", [4, 128], bf16, kind="ExternalInput").ap()
    v_d = nc.dram_tensor("v_out", [128, 512], f32, kind="ExternalOutput").ap()

    with tile.TileContext(nc) as tc:
        with (
            tc.tile_pool(name="wpool", bufs=1) as wpool,
            tc.tile_pool(name="state", bufs=1) as state,
            tc.tile_pool(name="scratch", bufs=2) as scratch,
            tc.tile_pool(name="smalls", bufs=2) as smalls,
            tc.tile_pool(name="psum", bufs=2, space="PSUM") as pp,
            tc.tile_pool(name="dram", bufs=2, space="DRAM") as dram,
        ):
            # ---- small inputs + WSN (pass A consumes tiles as they land) --
            utn = state.tile([NS, NH * D_IN * B], bf16, tag="utn")
            nc.sync.dma_start(utn[:], UTN_d[:])
            uu = state.tile([NH * B, NS * D_IN], bf16, tag="uu")
            nc.sync.dma_start(uu[:], UU_d[:])
            i128 = state.tile([128, 128], bf16, tag="i128")
            nc.sync.dma_start(i128[:], I128_d[:])
            j4 = state.tile([128, 4], bf16, tag="j4")
            nc.sync.dma_start(j4[:], J4_d[:])
            r4 = state.tile([4, 128], bf16, tag="r4")
            nc.sync.dma_start(r4[:], R4_d[:])
            wsn = []
            for t in range(32):
                wt = wpool.tile([NS, J], bf16, tag=f"wsn{t}")
                nc.sync.dma_start(wt[:], WSN_d[t])
                wsn.append(wt)

            blog = state.tile([128, N_OUT * NS], f32, tag="blog")
            nc.gpsimd.memset(blog[:], 0.0)
            vv = state.tile([128, NH * OG * B], bf16, tag="vv")

            # ---- pass A: sT1 = (1/32)*sum_n uhat, interleaved with DMA ----
            # og-quad q uses psum banks stj[0..3] cols [64*q : 64*q+64];
            # 4 og-groups accumulate interleaved in 4 distinct banks.
            psA = [pp.tile([128, 512], f32, tag="stj", bufs=4,
                           name=f"psA{tj}") for tj in range(4)]
            for q in range(2):  # og = q*4 + tj
                for nh in range(NH):
                    for i in range(D_IN):
                        for tj in range(4):
                            og = q * 4 + tj
                            nc.tensor.matmul(
                                psA[tj][:, q * 256 + tj * B:
                                        q * 256 + (tj + 1) * B],
                                lhsT=wsn[nh * D_IN + i][:, og * 128:
                                                        (og + 1) * 128],
                                rhs=utn[:, (nh * D_IN + i) * B:
                                        (nh * D_IN + i + 1) * B],
                                start=(nh == 0 and i == 0),
                                stop=(nh == 1 and i == D_IN - 1))

            # WG deferred: first needed after AR1
            wg = state.tile([128, OG * NH * NS * D_IN], bf16, tag="wg")
            for og in range(OG):
                sl = slice(og * NH * NS * D_IN, (og + 1) * NH * NS * D_IN)
                nc.sync.dma_start(wg[:, sl], WG_d[:, sl])

            def ar_squash(st_parts, scale, last):
                """st_parts: list of (psum_tile, part_slice, col_slice,
                out_col) -> evac+scale, AllReduce (bf16), squash into vv."""
                if last:
                    s_f = scratch.tile([128, 512], f32, tag="s_f", bufs=1)
                    for (t, psl, csl, oc) in st_parts:
                        nc.scalar.mul(s_f[psl, oc:oc + (csl.stop - csl.start)],
                                      t[psl, csl], scale)
                    nc.sync.dma_start(v_d[:], s_f[:])
                    return
                s_sb = scratch.tile([128, 512], bf16, tag="s_sb", bufs=1)
                for (t, psl, csl, oc) in st_parts:
                    nc.scalar.mul(s_sb[psl, oc:oc + (csl.stop - csl.start)],
                                  t[psl, csl], scale)
                bin_ = dram.tile([128, 512], bf16, tag="bounce_in")
                bout = dram.tile([128, 512], bf16, tag="bounce_out")
                nc.sync.dma_start(bin_[:], s_sb[:])
                nc.gpsimd.collective_compute(
                    "AllReduce", ALU.add,
                    replica_groups=[list(range(N_CORES))],
                    ins=[bin_.opt()], outs=[bout.opt()],
                )
                sTr = scratch.tile([128, 512], bf16, tag="sTr", bufs=1)
                nc.sync.dma_start(sTr[:], bout[:])
                # squash: v = s * sqrt(n2)/(1+n2), n2 = sum_k s^2
                sq = scratch.tile([128, 512], bf16, tag="sqq", bufs=1)
                nc.vector.tensor_mul(sq[:], sTr[:], sTr[:])
                n2 = pp.tile([4, 512], f32, tag="stj", bufs=4)
                nc.tensor.matmul(n2[:], lhsT=j4[:], rhs=sq[:],
                                 start=True, stop=True)
                tt = smalls.tile([4, 512], f32, tag="tt", bufs=1)
                nc.scalar.activation(tt[:], n2[:], AF.Sqrt)
                dd = smalls.tile([4, 512], f32, tag="dd", bufs=1)
                nc.scalar.add(dd[:], n2[:], 1.0)
                rr_ = smalls.tile([4, 512], f32, tag="rr", bufs=1)
                nc.vector.reciprocal(rr_[:], dd[:])
                scl = smalls.tile([4, 512], bf16, tag="scl", bufs=1)
                nc.vector.tensor_mul(scl[:], tt[:], rr_[:])
                rep = pp.tile([128, 512], f32, tag="stj", bufs=4)
                nc.tensor.matmul(rep[:], lhsT=r4[:], rhs=scl[:],
                                 start=True, stop=True)
                for nh in range(NH):
                    nc.vector.tensor_mul(
                        vv[:, nh * 512:(nh + 1) * 512], sTr[:], rep[:])

            passA_parts = []
            for og in range(OG):
                q, tj = divmod(og, 4)
                passA_parts.append(
                    (psA[tj], slice(0, 128),
                     slice(q * 256 + tj * B, q * 256 + (tj + 1) * B), og * B))
            ar_squash(passA_parts, 1.0 / N_OUT, last=False)

            # ---- routing iterations ----
            for r in (1, 2):
                # phase A: g + agreement -> blog
                for o in range(N_OUT):
                    og, jj = divmod(o, 4)
                    a_o = smalls.tile([128, NS], bf16, tag="a_o")
                    for h in range(2):  # (ns,i)-halves of 1024 cols
                        g = pp.tile([128, 1024], f32, tag="g", bufs=2)
                        for cc in range(2):
                            for nh in range(NH):
                                lhs = vv[32 * jj:32 * jj + 32,
                                         nh * 512 + og * B:
                                         nh * 512 + (og + 1) * B]
                                base = (og * NH * NS * D_IN
                                        + nh * NS * D_IN + h * 1024 + cc * 512)
                                nc.tensor.matmul(
                                    g[B * nh:B * (nh + 1),
                                      cc * 512:(cc + 1) * 512],
                                    lhsT=lhs,
                                    rhs=wg[32 * jj:32 * jj + 32,
                                           base:base + 512],
                                    start=True, stop=True,
                                    tile_position=(32 * jj, B * nh))
                        # evac on ACT (bf16), mul on VE/GpSimd, reduce on VE
                        ge = scratch.tile([128, 1024], bf16, tag="ge")
                        nc.scalar.mul(ge[:], g[:], 1.0)
                        th = scratch.tile([128, 1024], bf16, tag="th")
                        meng = nc.gpsimd if (h == 1 and o % 2 == 1) else nc.vector
                        meng.tensor_mul(
                            th[:], uu[:, h * 1024:(h + 1) * 1024], ge[:])
                        with nc.allow_low_precision(reason="a sums 16 terms"):
                            nc.vector.reduce_sum(
                                a_o[:, h * B:(h + 1) * B],
                                th[:].rearrange("p (ns i) -> p ns i", i=D_IN),
                                axis=AX.X)
                    nc.gpsimd.tensor_add(
                        blog[:, o * NS:(o + 1) * NS],
                        blog[:, o * NS:(o + 1) * NS], a_o[:])

                # phase B: softmax over o (no max-sub: |logits| < ~1).
                # denominator via contiguous tree-adds; dummy transposes
                # keep the PE HAM-warm through this VE/ACT-only stretch.
                ee = state.tile([128, N_OUT * NS], bf16, tag="ee")
                nc.scalar.activation(ee[:], blog[:], AF.Exp)
                tr = scratch.tile([128, 2048], bf16, tag="tree", bufs=1)
                nc.vector.tensor_add(tr[:], ee[:, 0:2048], ee[:, 2048:4096])
                trash = pp.tile([128, 512], bf16, tag="g", bufs=2,
                                name=f"trash{r}")
                nc.tensor.transpose(trash[:, 0:128], tr[:, 0:128], i128[:])
                w_ = 1024
                while w_ >= NS:
                    nc.vector.tensor_add(tr[:, 0:w_], tr[:, 0:w_],
                                         tr[:, w_:2 * w_])
                    if w_ in (512, NS):
                        nc.tensor.transpose(trash[:, 128:256], tr[:, 0:128],
                                            i128[:])
                    w_ //= 2
                rc = smalls.tile([128, NS], bf16, tag="rc")
                with nc.allow_low_precision(reason="softmax denom, tol 2e-2"):
                    nc.vector.reciprocal(rc[:], tr[:, 0:NS])
                nc.vector.tensor_mul(
                    ee[:].rearrange("p (o ns) -> p o ns", ns=NS),
                    ee[:].rearrange("p (o ns) -> p o ns", ns=NS),
                    rc[:].unsqueeze(1).broadcast_to([128, N_OUT, NS]))

                # phase C: transpose c, X = cT*u, sT-MMs (col-tiled over jj)
                last = (r == 2)
                stj = [pp.tile([128, 512], f32, tag="stj", bufs=4,
                               name=f"stj{r}_{jj}")
                       for jj in range(4)]
                for og in range(OG):
                    ct_ps = pp.tile([128, 512], bf16, tag="g", bufs=2)
                    for jj in range(4):
                        o = og * 4 + jj
                        nc.tensor.transpose(
                            ct_ps[:, jj * 128:(jj + 1) * 128],
                            ee[:, o * NS:(o + 1) * NS], i128[:])
                    ct = scratch.tile([128, 512], bf16, tag="ct")
                    nc.scalar.mul(ct[:], ct_ps[:], 1.0)
                    for nh in range(NH):
                        xts = []
                        for jj in range(4):
                            xt = scratch.tile([128, D_IN * B], bf16,
                                              tag="xt", bufs=8)
                            eng = nc.gpsimd if jj >= 2 else nc.vector
                            eng.tensor_mul(
                                xt[:].rearrange("p (i b) -> p i b", b=B),
                                utn[:, nh * D_IN * B:(nh + 1) * D_IN * B]
                                .rearrange("p (i b) -> p i b", b=B),
                                ct[:, jj * 128 + nh * B:jj * 128 + (nh + 1) * B]
                                .unsqueeze(1).broadcast_to([128, D_IN, B]))
                            xts.append(xt)
                        for i in range(D_IN):
                            for jj in range(4):
                                o = og * 4 + jj
                                nc.tensor.matmul(
                                    stj[jj][32 * jj:32 * jj + 32,
                                            og * B:(og + 1) * B],
                                    lhsT=wsn[nh * D_IN + i][:, o * 32:
                                                            (o + 1) * 32],
                                    rhs=xts[jj][:, i * B:(i + 1) * B],
                                    start=(nh == 0 and i == 0),
                                    stop=(nh == 1 and i == D_IN - 1),
                                    tile_position=(0, 32 * jj))
                ar_squash(
                    [(stj[jj], slice(32 * jj, 32 * jj + 32),
                      slice(0, 512), 0) for jj in range(4)],
                    1.0, last=last)

    nc.compile()
    return nc


def _get_program():
    if "nc" not in _CACHE:
        _CACHE["nc"] = _build_program()
    return _CACHE["nc"]


def _squash_np(s, axis=-1):
    n2 = np.sum(s * s, axis=axis, keepdims=True)
    return s * (n2 / (1.0 + n2) / np.sqrt(n2))


def kernel(u, W):
    from concourse.bass_utils import run_bass_kernel_spmd

    nc = _get_program()
    in_maps = _pack_inputs(np.asarray(u, np.float32), np.asarray(W, np.float32))
    res = run_bass_kernel_spmd(nc, in_maps, list(range(N_CORES)))
    # sum partial sT3 over cores; unpack sT layout; squash on host
    sT = np.zeros((128, 512), np.float64)
    for rm in res.results:
        sT += rm["v_out"].astype(np.float64)
    # sT[(j,k), (og,b)] = s3[b, 4*og+j, k]
    s = sT.reshape(4, 32, OG, B).transpose(3, 2, 0, 1).reshape(B, N_OUT, D_OUT)
    return _squash_np(s.astype(np.float32))


# revision 8
# speedup vs baseline: 1.9096x; 1.0418x over previous
"""Trainium2 Bass kernel for capsule-network dynamic routing (g-form).

See kernel_v2_632us.py docstring for the math.  v4 changes:
  - WG/UU columns per (og,nh) reordered to (i16, ns128) so the agreement
    i-reduction becomes 4 contiguous in-place tree-adds (VE or GpSimd)
    instead of a segmented tensor_reduce
  - phase A split per-o: even o = VE muls fused from PSUM; odd o = ACT
    evac + GpSimd muls/tree
  - blog kept in bf16
  - phase C: X-muls all on VE (GpSimd+VE concurrent broadcast muls
    contend on SBUF ports); c-transposes pipelined one og ahead so the
    PE has no og-boundary bubble
  - AllReduce payload in bf16

Layouts per core (all bf16):
  WSN [32=(nh,i), 128=ns, 1024=(o,k)] : W[nh*128+ns, i, (o,k)]
  WG  [128=(j,k), (og8,nh2,i16,ns128)]: W[nh*128+ns, i, (4og+j)*32+k]
  UTN [128=ns, (nh2,i16,b64)]         : u[b, nh*128+ns, i]
  UU2 [128=(nh,b), (i16,ns128)]       : u[b, nh*128+ns, i]
  vv  [128=(j,k), (nh2,og8,b64)]      : v[b, 4og+j, k]  (dup over nh)
"""

import numpy as np

B = 64
N_FULL = 2048
D_IN = 16
N_OUT = 32
D_OUT = 32
J = N_OUT * D_OUT  # 1024
N_CORES = 8
NL = N_FULL // N_CORES  # 256
NH = 2
NS = 128
OG = 8  # o-groups of 4

_CACHE = {}


def _pack_inputs(u, W):
    import ml_dtypes
    bf = ml_dtypes.bfloat16
    I128 = np.eye(128, dtype=np.float32).astype(bf)
    J4 = np.kron(np.eye(4, dtype=np.float32), np.ones((32, 1), np.float32)).astype(bf)
    R4 = np.ascontiguousarray(J4.T).astype(bf)
    in_maps = []
    for c in range(N_CORES):
        ul = u[:, c * NL:(c + 1) * NL, :]          # [64, 256, 16]
        Wl = W[c * NL:(c + 1) * NL]                # [256, 16, 1024]
        WSN = np.ascontiguousarray(
            Wl.reshape(NH, NS, D_IN, J).transpose(0, 2, 1, 3)
            .reshape(NH * D_IN, NS, J)).astype(bf)
        # WG[(j,k), og, nh, i, ns]
        WG = np.ascontiguousarray(
            Wl.reshape(NH, NS, D_IN, OG, 4, 32).transpose(4, 5, 3, 0, 2, 1)
            .reshape(128, OG * NH * D_IN * NS)).astype(bf)
        UTN = np.ascontiguousarray(
            ul.reshape(B, NH, NS, D_IN).transpose(2, 1, 3, 0)
            .reshape(NS, NH * D_IN * B)).astype(bf)
        # UU2[(nh,b), (i,ns)]
        UU2 = np.ascontiguousarray(
            ul.reshape(B, NH, NS, D_IN).transpose(1, 0, 3, 2)
            .reshape(NH * B, D_IN * NS)).astype(bf)
        in_maps.append({"WSN": WSN, "WG": WG, "UTN": UTN, "UU2": UU2,
                        "I128": I128, "J4": J4, "R4": R4})
    return in_maps


def _build_program():
    import concourse.bass as bass  # noqa: F401
    import concourse.tile as tile
    from concourse import bacc, mybir

    f32 = mybir.dt.float32
    bf16 = mybir.dt.bfloat16
    AF = mybir.ActivationFunctionType
    ALU = mybir.AluOpType

    nc = bacc.Bacc("TRN2", target_bir_lowering=False, debug=False,
                   num_devices=N_CORES)
    WSN_d = nc.dram_tensor("WSN", [32, NS, J], bf16, kind="ExternalInput").ap()
    WG_d = nc.dram_tensor("WG", [128, OG * NH * NS * D_IN], bf16,
                          kind="ExternalInput").ap()
    UTN_d = nc.dram_tensor("UTN", [NS, NH * D_IN * B], bf16,
                           kind="ExternalInput").ap()
    UU2_d = nc.dram_tensor("UU2", [NH * B, NS * D_IN], bf16,
                           kind="ExternalInput").ap()
    I128_d = nc.dram_tensor("I128", [128, 128], bf16, kind="ExternalInput").ap()
    J4_d = nc.dram_tensor("J4", [128, 4], bf16, kind="ExternalInput").ap()
    R4_d = nc.dram_tensor("R4", [4, 128], bf16, kind="ExternalInput").ap()
    v_d = nc.dram_tensor("v_out", [128, 512], f32, kind="ExternalOutput").ap()

    with tile.TileContext(nc) as tc:
        with (
            tc.tile_pool(name="wpool", bufs=1) as wpool,
            tc.tile_pool(name="state", bufs=1) as state,
            tc.tile_pool(name="scratch", bufs=2) as scratch,
            tc.tile_pool(name="smalls", bufs=2) as smalls,
            tc.tile_pool(name="psum", bufs=2, space="PSUM") as pp,
            tc.tile_pool(name="dram", bufs=2, space="DRAM") as dram,
        ):
            # ---- small inputs + WSN (pass A consumes tiles as they land) --
            utn = state.tile([NS, NH * D_IN * B], bf16, tag="utn")
            nc.sync.dma_start(utn[:], UTN_d[:])
            uu2 = state.tile([NH * B, NS * D_IN], bf16, tag="uu2")
            nc.sync.dma_start(uu2[:], UU2_d[:])
            i128 = state.tile([128, 128], bf16, tag="i128")
            nc.sync.dma_start(i128[:], I128_d[:])
            j4 = state.tile([128, 4], bf16, tag="j4")
            nc.sync.dma_start(j4[:], J4_d[:])
            r4 = state.tile([4, 128], bf16, tag="r4")
            nc.sync.dma_start(r4[:], R4_d[:])
            wsn = []
            for t in range(32):
                wt = wpool.tile([NS, J], bf16, tag=f"wsn{t}")
                nc.sync.dma_start(wt[:], WSN_d[t])
                wsn.append(wt)

            blog = state.tile([128, N_OUT * NS], bf16, tag="blog")
            nc.gpsimd.memset(blog[:], 0.0)
            vv = state.tile([128, NH * OG * B], bf16, tag="vv")

            # ---- pass A: sT1 = (1/32)*sum_n uhat, interleaved with DMA ----
            psA = [pp.tile([128, 512], f32, tag="stj", bufs=4,
                           name=f"psA{tj}") for tj in range(4)]
            for q in range(2):  # og = q*4 + tj
                for nh in range(NH):
                    for i in range(D_IN):
                        for tj in range(4):
                            og = q * 4 + tj
                            nc.tensor.matmul(
                                psA[tj][:, q * 256 + tj * B:
                                        q * 256 + (tj + 1) * B],
                                lhsT=wsn[nh * D_IN + i][:, og * 128:
                                                        (og + 1) * 128],
                                rhs=utn[:, (nh * D_IN + i) * B:
                                        (nh * D_IN + i + 1) * B],
                                start=(nh == 0 and i == 0),
                                stop=(nh == 1 and i == D_IN - 1))

            # WG deferred: first needed after AR1
            wg = state.tile([128, OG * NH * NS * D_IN], bf16, tag="wg")
            for og in range(OG):
                sl = slice(og * NH * NS * D_IN, (og + 1) * NH * NS * D_IN)
                nc.sync.dma_start(wg[:, sl], WG_d[:, sl])

            def ar_squash(st_parts, scale, last):
                if last:
                    s_f = scratch.tile([128, 512], f32, tag="s_f", bufs=1)
                    for (t, psl, csl, oc) in st_parts:
                        nc.scalar.mul(s_f[psl, oc:oc + (csl.stop - csl.start)],
                                      t[psl, csl], scale)
                    nc.sync.dma_start(v_d[:], s_f[:])
                    return
                s_sb = scratch.tile([128, 512], bf16, tag="s_sb", bufs=1)
                for (t, psl, csl, oc) in st_parts:
                    nc.scalar.mul(s_sb[psl, oc:oc + (csl.stop - csl.start)],
                                  t[psl, csl], scale)
                bin_ = dram.tile([128, 512], bf16, tag="bounce_in")
                bout = dram.tile([128, 512], bf16, tag="bounce_out")
                nc.sync.dma_start(bin_[:], s_sb[:])
                nc.gpsimd.collective_compute(
                    "AllReduce", ALU.add,
                    replica_groups=[list(range(N_CORES))],
                    ins=[bin_.opt()], outs=[bout.opt()],
                )
                sTr = scratch.tile([128, 512], bf16, tag="sTr", bufs=1)
                nc.sync.dma_start(sTr[:], bout[:])
                sq = scratch.tile([128, 512], bf16, tag="sqq", bufs=1)
                nc.vector.tensor_mul(sq[:], sTr[:], sTr[:])
                n2 = pp.tile([4, 512], f32, tag="stj", bufs=4)
                nc.tensor.matmul(n2[:], lhsT=j4[:], rhs=sq[:],
                                 start=True, stop=True)
                tt = smalls.tile([4, 512], f32, tag="tt", bufs=1)
                nc.scalar.activation(tt[:], n2[:], AF.Sqrt)
                dd = smalls.tile([4, 512], f32, tag="dd", bufs=1)
                nc.scalar.add(dd[:], n2[:], 1.0)
                rr_ = smalls.tile([4, 512], f32, tag="rr", bufs=1)
                nc.vector.reciprocal(rr_[:], dd[:])
                scl = smalls.tile([4, 512], bf16, tag="scl", bufs=1)
                nc.vector.tensor_mul(scl[:], tt[:], rr_[:])
                rep = pp.tile([128, 512], f32, tag="stj", bufs=4)
                nc.tensor.matmul(rep[:], lhsT=r4[:], rhs=scl[:],
                                 start=True, stop=True)
                for nh in range(NH):
                    nc.vector.tensor_mul(
                        vv[:, nh * 512:(nh + 1) * 512], sTr[:], rep[:])

            passA_parts = []
            for og in range(OG):
                q, tj = divmod(og, 4)
                passA_parts.append(
                    (psA[tj], slice(0, 128),
                     slice(q * 256 + tj * B, q * 256 + (tj + 1) * B), og * B))
            ar_squash(passA_parts, 1.0 / N_OUT, last=False)

            # ---- routing iterations ----
            for r in (1, 2):
                # phase A: g + agreement -> blog
                # even o: VE muls fused from PSUM + VE tree
                # odd o:  ACT evac + GpSimd muls + GpSimd tree
                for o in range(N_OUT):
                    og, jj = divmod(o, 4)
                    on_ve = (o % 2 == 0)
                    th = scratch.tile([128, 2048], bf16, tag="th")
                    for h in range(2):  # i-halves: i in [8h, 8h+8)
                        g = pp.tile([128, 1024], f32, tag="g", bufs=2)
                        for cc in range(2):
                            for nh in range(NH):
                                lhs = vv[32 * jj:32 * jj + 32,
                                         nh * 512 + og * B:
                                         nh * 512 + (og + 1) * B]
                                base = (og * NH * NS * D_IN
                                        + nh * NS * D_IN + h * 1024 + cc * 512)
                                nc.tensor.matmul(
                                    g[B * nh:B * (nh + 1),
                                      cc * 512:(cc + 1) * 512],
                                    lhsT=lhs,
                                    rhs=wg[32 * jj:32 * jj + 32,
                                           base:base + 512],
                                    start=True, stop=True,
                                    tile_position=(32 * jj, B * nh))
                        if on_ve:
                            nc.vector.tensor_mul(
                                th[:, h * 1024:(h + 1) * 1024],
                                uu2[:, h * 1024:(h + 1) * 1024], g[:])
                        else:
                            ge = scratch.tile([128, 1024], bf16, tag="ge")
                            nc.scalar.mul(ge[:], g[:], 1.0)
                            nc.gpsimd.tensor_mul(
                                th[:, h * 1024:(h + 1) * 1024],
                                uu2[:, h * 1024:(h + 1) * 1024], ge[:])
                    # tree-reduce over i (contiguous halves), in place
                    teng = nc.vector if on_ve else nc.gpsimd
                    w_ = 1024
                    while w_ >= NS:
                        teng.tensor_add(th[:, 0:w_], th[:, 0:w_],
                                        th[:, w_:2 * w_])
                        w_ //= 2
                    aeng = nc.vector if on_ve else nc.gpsimd
                    aeng.tensor_add(
                        blog[:, o * NS:(o + 1) * NS],
                        blog[:, o * NS:(o + 1) * NS], th[:, 0:NS])

                # phase B: softmax over o (no max-sub: |logits| < ~1)
                ee = state.tile([128, N_OUT * NS], bf16, tag="ee")
                nc.scalar.activation(ee[:], blog[:], AF.Exp)
                tr = scratch.tile([128, 2048], bf16, tag="tree", bufs=1)
                nc.vector.tensor_add(tr[:], ee[:, 0:2048], ee[:, 2048:4096])
                w_ = 1024
                while w_ >= NS:
                    nc.vector.tensor_add(tr[:, 0:w_], tr[:, 0:w_],
                                         tr[:, w_:2 * w_])
                    w_ //= 2
                rc = smalls.tile([128, NS], bf16, tag="rc")
                with nc.allow_low_precision(reason="softmax denom, tol 2e-2"):
                    nc.vector.reciprocal(rc[:], tr[:, 0:NS])
                nc.vector.tensor_mul(
                    ee[:].rearrange("p (o ns) -> p o ns", ns=NS),
                    ee[:].rearrange("p (o ns) -> p o ns", ns=NS),
                    rc[:].unsqueeze(1).broadcast_to([128, N_OUT, NS]))

                # phase C: transpose c, X = cT*u (VE only), sT-MMs
                # (col-tiled over jj; og+1 transposes pipelined early)
                last = (r == 2)
                stj = [pp.tile([128, 512], f32, tag="stj", bufs=4,
                               name=f"stj{r}_{jj}")
                       for jj in range(4)]

                ct_ps = {}
                ct = {}

                def emit_transposes(g_):
                    ct_ps[g_] = pp.tile([128, 512], bf16, tag="g", bufs=2,
                                        name=f"ctps{r}_{g_}")
                    for j_ in range(4):
                        o_ = g_ * 4 + j_
                        nc.tensor.transpose(
                            ct_ps[g_][:, j_ * 128:(j_ + 1) * 128],
                            ee[:, o_ * NS:(o_ + 1) * NS], i128[:])
                    ct[g_] = scratch.tile([128, 512], bf16, tag="ct",
                                          name=f"ct{r}_{g_}")
                    nc.scalar.mul(ct[g_][:], ct_ps[g_][:], 1.0)

                emit_transposes(0)
                for og in range(OG):
                    for nh in range(NH):
                        xts = []
                        for jj in range(4):
                            xt = scratch.tile([128, D_IN * B], bf16,
                                              tag="xt", bufs=10)
                            nc.vector.tensor_mul(
                                xt[:].rearrange("p (i b) -> p i b", b=B),
                                utn[:, nh * D_IN * B:(nh + 1) * D_IN * B]
                                .rearrange("p (i b) -> p i b", b=B),
                                ct[og][:, jj * 128 + nh * B:
                                        jj * 128 + (nh + 1) * B]
                                .unsqueeze(1).broadcast_to([128, D_IN, B]))
                            xts.append(xt)
                        if nh == 0 and og + 1 < OG:
                            emit_transposes(og + 1)
                        for i in range(D_IN):
                            for jj in range(4):
                                o = og * 4 + jj
                                nc.tensor.matmul(
                                    stj[jj][32 * jj:32 * jj + 32,
                                            og * B:(og + 1) * B],
                                    lhsT=wsn[nh * D_IN + i][:, o * 32:
                                                            (o + 1) * 32],
                                    rhs=xts[jj][:, i * B:(i + 1) * B],
                                    start=(nh == 0 and i == 0),
                                    stop=(nh == 1 and i == D_IN - 1),
                                    tile_position=(0, 32 * jj))
                ar_squash(
                    [(stj[jj], slice(32 * jj, 32 * jj + 32),
                      slice(0, 512), 0) for jj in range(4)],
                    1.0, last=last)

    nc.compile()
    return nc


def _get_program():
    if "nc" not in _CACHE:
        _CACHE["nc"] = _build_program()
    return _CACHE["nc"]


def _squash_np(s, axis=-1):
    n2 = np.sum(s * s, axis=axis, keepdims=True)
    return s * (n2 / (1.0 + n2) / np.sqrt(n2))


def kernel(u, W):
    from concourse.bass_utils import run_bass_kernel_spmd

    nc = _get_program()
    in_maps = _pack_inputs(np.asarray(u, np.float32), np.asarray(W, np.float32))
    res = run_bass_kernel_spmd(nc, in_maps, list(range(N_CORES)))
    sT = np.zeros((128, 512), np.float64)
    for rm in res.results:
        sT += rm["v_out"].astype(np.float64)
    # sT[(j,k), (og,b)] = s3[b, 4*og+j, k]
    s = sT.reshape(4, 32, OG, B).transpose(3, 2, 0, 1).reshape(B, N_OUT, D_OUT)
    return _squash_np(s.astype(np.float32))


# revision 9
# speedup vs baseline: 2.4726x; 1.2948x over previous
"""Trainium2 Bass kernel for capsule-network dynamic routing (g-form).

See kernel_v2_632us.py docstring for the math.  v4 changes:
  - WG/UU columns per (og,nh) reordered to (i16, ns128) so the agreement
    i-reduction becomes 4 contiguous in-place tree-adds (VE or GpSimd)
    instead of a segmented tensor_reduce
  - phase A split per-o: even o = VE muls fused from PSUM; odd o = ACT
    evac + GpSimd muls/tree
  - blog kept in bf16
  - phase C: X-muls all on VE (GpSimd+VE concurrent broadcast muls
    contend on SBUF ports); c-transposes pipelined one og ahead so the
    PE has no og-boundary bubble
  - AllReduce payload in bf16

Layouts per core (all bf16):
  WSN [32=(nh,i), 128=ns, 1024=(o,k)] : W[nh*128+ns, i, (o,k)]
  WG  [128=(j,k), (og8,nh2,i16,ns128)]: W[nh*128+ns, i, (4og+j)*32+k]
  UTN [128=ns, (nh2,i16,b64)]         : u[b, nh*128+ns, i]
  UU2 [128=(nh,b), (i16,ns128)]       : u[b, nh*128+ns, i]
  vv  [128=(j,k), (nh2,og8,b64)]      : v[b, 4og+j, k]  (dup over nh)
"""

import numpy as np

B = 64
N_FULL = 2048
D_IN = 16
N_OUT = 32
D_OUT = 32
J = N_OUT * D_OUT  # 1024
N_CORES = 8
NL = N_FULL // N_CORES  # 256
NH = 2
NS = 128
OG = 8  # o-groups of 4

_CACHE = {}


def _pack_inputs(u, W):
    import ml_dtypes
    bf = ml_dtypes.bfloat16
    I128 = np.eye(128, dtype=np.float32).astype(bf)
    J4 = np.kron(np.eye(4, dtype=np.float32), np.ones((32, 1), np.float32)).astype(bf)
    R4 = np.ascontiguousarray(J4.T).astype(bf)
    in_maps = []
    for c in range(N_CORES):
        ul = u[:, c * NL:(c + 1) * NL, :]          # [64, 256, 16]
        Wl = W[c * NL:(c + 1) * NL]                # [256, 16, 1024]
        WSN = np.ascontiguousarray(
            Wl.reshape(NH, NS, D_IN, J).transpose(0, 2, 1, 3)
            .reshape(NH * D_IN, NS, J)).astype(bf)
        # WG[(j,k), og, nh, i, ns]
        WG = np.ascontiguousarray(
            Wl.reshape(NH, NS, D_IN, OG, 4, 32).transpose(4, 5, 3, 0, 2, 1)
            .reshape(128, OG * NH * D_IN * NS)).astype(bf)
        UTN = np.ascontiguousarray(
            ul.reshape(B, NH, NS, D_IN).transpose(2, 1, 3, 0)
            .reshape(NS, NH * D_IN * B)).astype(bf)
        # UU2[(nh,b), (i,ns)]
        UU2 = np.ascontiguousarray(
            ul.reshape(B, NH, NS, D_IN).transpose(1, 0, 3, 2)
            .reshape(NH * B, D_IN * NS)).astype(bf)
        in_maps.append({"WSN": WSN, "WG": WG, "UTN": UTN, "UU2": UU2,
                        "I128": I128, "J4": J4, "R4": R4})
    return in_maps


def _build_program():
    import concourse.bass as bass  # noqa: F401
    import concourse.tile as tile
    from concourse import bacc, mybir

    f32 = mybir.dt.float32
    bf16 = mybir.dt.bfloat16
    AF = mybir.ActivationFunctionType
    ALU = mybir.AluOpType

    nc = bacc.Bacc("TRN2", target_bir_lowering=False, debug=False,
                   num_devices=N_CORES)
    WSN_d = nc.dram_tensor("WSN", [32, NS, J], bf16, kind="ExternalInput").ap()
    WG_d = nc.dram_tensor("WG", [128, OG * NH * NS * D_IN], bf16,
                          kind="ExternalInput").ap()
    UTN_d = nc.dram_tensor("UTN", [NS, NH * D_IN * B], bf16,
                           kind="ExternalInput").ap()
    UU2_d = nc.dram_tensor("UU2", [NH * B, NS * D_IN], bf16,
                           kind="ExternalInput").ap()
    I128_d = nc.dram_tensor("I128", [128, 128], bf16, kind="ExternalInput").ap()
    J4_d = nc.dram_tensor("J4", [128, 4], bf16, kind="ExternalInput").ap()
    R4_d = nc.dram_tensor("R4", [4, 128], bf16, kind="ExternalInput").ap()
    v_d = nc.dram_tensor("v_out", [128, 512], f32, kind="ExternalOutput").ap()

    with tile.TileContext(nc) as tc:
        with (
            tc.tile_pool(name="wpool", bufs=1) as wpool,
            tc.tile_pool(name="state", bufs=1) as state,
            tc.tile_pool(name="scratch", bufs=2) as scratch,
            tc.tile_pool(name="smalls", bufs=2) as smalls,
            tc.tile_pool(name="psum", bufs=2, space="PSUM") as pp,
            tc.tile_pool(name="dram", bufs=2, space="DRAM") as dram,
        ):
            # ---- small inputs + WSN (pass A consumes tiles as they land) --
            utn = state.tile([NS, NH * D_IN * B], bf16, tag="utn")
            nc.sync.dma_start(utn[:], UTN_d[:])
            uu2 = state.tile([NH * B, NS * D_IN], bf16, tag="uu2")
            nc.sync.dma_start(uu2[:], UU2_d[:])
            i128 = state.tile([128, 128], bf16, tag="i128")
            nc.sync.dma_start(i128[:], I128_d[:])
            j4 = state.tile([128, 4], bf16, tag="j4")
            nc.sync.dma_start(j4[:], J4_d[:])
            r4 = state.tile([4, 128], bf16, tag="r4")
            nc.sync.dma_start(r4[:], R4_d[:])
            wsn = []
            for t in range(32):
                wt = wpool.tile([NS, J], bf16, tag=f"wsn{t}")
                nc.sync.dma_start(wt[:], WSN_d[t])
                wsn.append(wt)

            blog = state.tile([128, N_OUT * NS], bf16, tag="blog")
            nc.gpsimd.memset(blog[:], 0.0)
            vv = state.tile([128, NH * OG * B], bf16, tag="vv")

            # ---- pass A: sT1 = (1/32)*sum_n uhat, interleaved with DMA ----
            psA = [pp.tile([128, 512], f32, tag="stj", bufs=4,
                           name=f"psA{tj}") for tj in range(4)]
            for q in range(2):  # og = q*4 + tj
                for nh in range(NH):
                    for i in range(D_IN):
                        for tj in range(4):
                            og = q * 4 + tj
                            nc.tensor.matmul(
                                psA[tj][:, q * 256 + tj * B:
                                        q * 256 + (tj + 1) * B],
                                lhsT=wsn[nh * D_IN + i][:, og * 128:
                                                        (og + 1) * 128],
                                rhs=utn[:, (nh * D_IN + i) * B:
                                        (nh * D_IN + i + 1) * B],
                                start=(nh == 0 and i == 0),
                                stop=(nh == 1 and i == D_IN - 1))

            # WG deferred: first needed after AR1
            wg = state.tile([128, OG * NH * NS * D_IN], bf16, tag="wg")
            for og in range(OG):
                sl = slice(og * NH * NS * D_IN, (og + 1) * NH * NS * D_IN)
                nc.sync.dma_start(wg[:, sl], WG_d[:, sl])

            def ar_squash(st_parts, scale, last):
                if last:
                    s_f = scratch.tile([128, 512], f32, tag="s_f", bufs=1)
                    for (t, psl, csl, oc) in st_parts:
                        nc.scalar.mul(s_f[psl, oc:oc + (csl.stop - csl.start)],
                                      t[psl, csl], scale)
                    nc.sync.dma_start(v_d[:], s_f[:])
                    return
                s_sb = scratch.tile([128, 512], bf16, tag="s_sb", bufs=1)
                for (t, psl, csl, oc) in st_parts:
                    nc.scalar.mul(s_sb[psl, oc:oc + (csl.stop - csl.start)],
                                  t[psl, csl], scale)
                bin_ = dram.tile([128, 512], bf16, tag="bounce_in")
                bout = dram.tile([128, 512], bf16, tag="bounce_out")
                nc.sync.dma_start(bin_[:], s_sb[:])
                nc.gpsimd.collective_compute(
                    "AllReduce", ALU.add,
                    replica_groups=[list(range(N_CORES))],
                    ins=[bin_.opt()], outs=[bout.opt()],
                )
                sTr = scratch.tile([128, 512], bf16, tag="sTr", bufs=1)
                nc.sync.dma_start(sTr[:], bout[:])
                sq = scratch.tile([128, 512], bf16, tag="sqq", bufs=1)
                nc.vector.tensor_mul(sq[:], sTr[:], sTr[:])
                n2 = pp.tile([4, 512], f32, tag="stj", bufs=4)
                nc.tensor.matmul(n2[:], lhsT=j4[:], rhs=sq[:],
                                 start=True, stop=True)
                tt = smalls.tile([4, 512], f32, tag="tt", bufs=1)
                nc.scalar.activation(tt[:], n2[:], AF.Sqrt)
                dd = smalls.tile([4, 512], f32, tag="dd", bufs=1)
                nc.scalar.add(dd[:], n2[:], 1.0)
                rr_ = smalls.tile([4, 512], f32, tag="rr", bufs=1)
                nc.vector.reciprocal(rr_[:], dd[:])
                scl = smalls.tile([4, 512], bf16, tag="scl", bufs=1)
                nc.vector.tensor_mul(scl[:], tt[:], rr_[:])
                rep = pp.tile([128, 512], f32, tag="stj", bufs=4)
                nc.tensor.matmul(rep[:], lhsT=r4[:], rhs=scl[:],
                                 start=True, stop=True)
                for nh in range(NH):
                    nc.vector.tensor_mul(
                        vv[:, nh * 512:(nh + 1) * 512], sTr[:], rep[:])

            passA_parts = []
            for og in range(OG):
                q, tj = divmod(og, 4)
                passA_parts.append(
                    (psA[tj], slice(0, 128),
                     slice(q * 256 + tj * B, q * 256 + (tj + 1) * B), og * B))
            ar_squash(passA_parts, 1.0 / N_OUT, last=False)

            # ---- routing iterations ----
            for r in (1, 2):
                # phase A: g + agreement -> blog
                # even o: VE muls fused from PSUM + VE tree
                # odd o:  ACT evac + GpSimd muls + GpSimd tree
                for o in range(N_OUT):
                    og, jj = divmod(o, 4)
                    th = scratch.tile([128, 2048], bf16, tag="th")
                    for h in range(2):  # i-halves: i in [8h, 8h+8)
                        g = pp.tile([128, 1024], f32, tag="g", bufs=2)
                        for cc in range(2):
                            for nh in range(NH):
                                lhs = vv[32 * jj:32 * jj + 32,
                                         nh * 512 + og * B:
                                         nh * 512 + (og + 1) * B]
                                base = (og * NH * NS * D_IN
                                        + nh * NS * D_IN + h * 1024 + cc * 512)
                                nc.tensor.matmul(
                                    g[B * nh:B * (nh + 1),
                                      cc * 512:(cc + 1) * 512],
                                    lhsT=lhs,
                                    rhs=wg[32 * jj:32 * jj + 32,
                                           base:base + 512],
                                    start=True, stop=True,
                                    tile_position=(32 * jj, B * nh))
                        # ACT evacuates; VE multiplies at 2x (GpSimd is
                        # kept idle: concurrent VE+GpSimd SBUF streaming
                        # degrades both ~3x)
                        ge = scratch.tile([128, 1024], bf16, tag="ge",
                                          bufs=3)
                        nc.scalar.mul(ge[:], g[:], 1.0)
                        nc.vector.tensor_mul(
                            th[:, h * 1024:(h + 1) * 1024],
                            uu2[:, h * 1024:(h + 1) * 1024], ge[:])
                    # tree-reduce over i (contiguous halves), in place
                    w_ = 1024
                    while w_ >= NS:
                        nc.vector.tensor_add(th[:, 0:w_], th[:, 0:w_],
                                             th[:, w_:2 * w_])
                        w_ //= 2
                    nc.gpsimd.tensor_add(
                        blog[:, o * NS:(o + 1) * NS],
                        blog[:, o * NS:(o + 1) * NS], th[:, 0:NS])

                # phase B: softmax over o (no max-sub: |logits| < ~1)
                ee = state.tile([128, N_OUT * NS], bf16, tag="ee")
                nc.scalar.activation(ee[:], blog[:], AF.Exp)
                tr = scratch.tile([128, 2048], bf16, tag="tree", bufs=1)
                nc.vector.tensor_add(tr[:], ee[:, 0:2048], ee[:, 2048:4096])
                w_ = 1024
                while w_ >= NS:
                    nc.vector.tensor_add(tr[:, 0:w_], tr[:, 0:w_],
                                         tr[:, w_:2 * w_])
                    w_ //= 2
                rc = smalls.tile([128, NS], bf16, tag="rc")
                with nc.allow_low_precision(reason="softmax denom, tol 2e-2"):
                    nc.vector.reciprocal(rc[:], tr[:, 0:NS])
                nc.vector.tensor_mul(
                    ee[:].rearrange("p (o ns) -> p o ns", ns=NS),
                    ee[:].rearrange("p (o ns) -> p o ns", ns=NS),
                    rc[:].unsqueeze(1).broadcast_to([128, N_OUT, NS]))

                # phase C: transpose c, X = cT*u (VE only), sT-MMs
                # (col-tiled over jj; og+1 transposes pipelined early)
                last = (r == 2)
                stj = [pp.tile([128, 512], f32, tag="stj", bufs=4,
                               name=f"stj{r}_{jj}")
                       for jj in range(4)]

                ct_ps = {}
                ct = {}

                def emit_transposes(g_):
                    ct_ps[g_] = pp.tile([128, 512], bf16, tag="g", bufs=2,
                                        name=f"ctps{r}_{g_}")
                    for j_ in range(4):
                        o_ = g_ * 4 + j_
                        nc.tensor.transpose(
                            ct_ps[g_][:, j_ * 128:(j_ + 1) * 128],
                            ee[:, o_ * NS:(o_ + 1) * NS], i128[:])
                    ct[g_] = scratch.tile([128, 512], bf16, tag="ct",
                                          name=f"ct{r}_{g_}")
                    nc.scalar.mul(ct[g_][:], ct_ps[g_][:], 1.0)

                emit_transposes(0)
                for og in range(OG):
                    for nh in range(NH):
                        xts = []
                        for jj in range(4):
                            xt = scratch.tile([128, D_IN * B], bf16,
                                              tag="xt", bufs=10)
                            nc.vector.tensor_mul(
                                xt[:].rearrange("p (i b) -> p i b", b=B),
                                utn[:, nh * D_IN * B:(nh + 1) * D_IN * B]
                                .rearrange("p (i b) -> p i b", b=B),
                                ct[og][:, jj * 128 + nh * B:
                                        jj * 128 + (nh + 1) * B]
                                .unsqueeze(1).broadcast_to([128, D_IN, B]))
                            xts.append(xt)
                        if nh == 0 and og + 1 < OG:
                            emit_transposes(og + 1)
                        for i in range(D_IN):
                            for jj in range(4):
                                o = og * 4 + jj
                                nc.tensor.matmul(
                                    stj[jj][32 * jj:32 * jj + 32,
                                            og * B:(og + 1) * B],
                                    lhsT=wsn[nh * D_IN + i][:, o * 32:
                                                            (o + 1) * 32],
                                    rhs=xts[jj][:, i * B:(i + 1) * B],
                                    start=(nh == 0 and i == 0),
                                    stop=(nh == 1 and i == D_IN - 1),
                                    tile_position=(0, 32 * jj))
                ar_squash(
                    [(stj[jj], slice(32 * jj, 32 * jj + 32),
                      slice(0, 512), 0) for jj in range(4)],
                    1.0, last=last)

    nc.compile()
    return nc


def _get_program():
    if "nc" not in _CACHE:
        _CACHE["nc"] = _build_program()
    return _CACHE["nc"]


def _squash_np(s, axis=-1):
    n2 = np.sum(s * s, axis=axis, keepdims=True)
    return s * (n2 / (1.0 + n2) / np.sqrt(n2))


def kernel(u, W):
    from concourse.bass_utils import run_bass_kernel_spmd

    nc = _get_program()
    in_maps = _pack_inputs(np.asarray(u, np.float32), np.asarray(W, np.float32))
    res = run_bass_kernel_spmd(nc, in_maps, list(range(N_CORES)))
    sT = np.zeros((128, 512), np.float64)
    for rm in res.results:
        sT += rm["v_out"].astype(np.float64)
    # sT[(j,k), (og,b)] = s3[b, 4*og+j, k]
    s = sT.reshape(4, 32, OG, B).transpose(3, 2, 0, 1).reshape(B, N_OUT, D_OUT)
    return _squash_np(s.astype(np.float32))
